# revision 1
# baseline (speedup 1.0000x reference)
"""Causal self-attention with RoPE for trn2, 8-core SPMD.

Sharding: core i handles batch b = i//2 and heads [8*(i%2), 8*(i%2)+8).
Each core computes a partial output [T, C] = y_local @ Wo_rows_local;
host sums core pairs and adds bo.

Layout strategy (per core, all matmuls in float32r):
  P1: qT/kT per head in [d, t] layout via lhsT=Wq tiles, rhs=xT tiles;
      bias added on ACT eviction; RoPE on DVE in even/odd-deinterleaved
      row order (weights column-permuted on host); v in [t, d] layout.
      q/k/v round-trip through internal DRAM.
  P2: per head flash-style: St[tk,tq] = kT_blk.T @ qT (K=d=128, single
      matmul), E = exp(scale*St) on ACT, binary-mask diag blocks on DVE,
      denom accumulated via ones-matmul (M=1), yT accumulated via
      lhsT=v_blk. Normalization folded into PSUM eviction using
      reciprocal + partition-broadcast DMA. No max-subtraction (scores
      are O(10), fp32 exp cannot overflow).
  P3: out[t, c] = sum_h yT_h[:, t-tile].T @ Wo rows.
"""
from contextlib import ExitStack

import numpy as np

import concourse.bacc as bacc
import concourse.tile as tile
from concourse import mybir

F32 = mybir.dt.float32
F32R = mybir.dt.float32r
AF = mybir.ActivationFunctionType
ALU = mybir.AluOpType

N_HEAD = 16
HEAD_DIM = 128
ROPE_BASE = 10000.0


def r32(ap):
    return ap


def build_core_kernel(T=2048, C=2048, HL=8, reps=1, pool_mode="stack"):
    """One core's program: full T, C channels, HL local heads."""
    D = HEAD_DIM
    CL = HL * D            # local q/k/v channels (1024)
    NCT = C // 128         # c-tiles (16)
    NQ = T // 512          # 512-wide t slices (4)
    NT = T // 128          # 128-wide t tiles (16)
    NG = CL // 512         # v column groups (2)
    NCQ = C // 512         # out-proj column groups (4)
    scale = 1.0 / float(np.sqrt(D))

    nc = bacc.Bacc("TRN2", target_bir_lowering=False, debug=False)

    xT_d = nc.dram_tensor("xT", [C, T], F32R, kind="ExternalInput")
    wq_d = nc.dram_tensor("wq", [C, CL], F32R, kind="ExternalInput")
    wk_d = nc.dram_tensor("wk", [C, CL], F32R, kind="ExternalInput")
    wv_d = nc.dram_tensor("wv", [C, CL], F32R, kind="ExternalInput")
    wo_d = nc.dram_tensor("wo", [CL, C], F32R, kind="ExternalInput")
    bq_d = nc.dram_tensor("bq", [CL], F32, kind="ExternalInput")
    bk_d = nc.dram_tensor("bk", [CL], F32, kind="ExternalInput")
    bv_d = nc.dram_tensor("bv", [CL], F32, kind="ExternalInput")
    cos_d = nc.dram_tensor("cos2", [128, T], F32, kind="ExternalInput")
    sin_d = nc.dram_tensor("sin2", [128, T], F32, kind="ExternalInput")
    mask_d = nc.dram_tensor("masks", [4, 128, 512], F32, kind="ExternalInput")
    ones_d = nc.dram_tensor("ones", [128, 128], F32R, kind="ExternalInput")
    out_d = nc.dram_tensor("out_p", [T, C], F32, kind="ExternalOutput")

    with tile.TileContext(nc, pool_alloc_mode=pool_mode) as tc, ExitStack() as top:
        dram = top.enter_context(tc.tile_pool(name="dram", bufs=1, space="DRAM"))
        q_rt = dram.tile([HL, 128, T], F32R)
        k_rt = dram.tile([HL, 128, T], F32R)
        v_rt = dram.tile([NT, NG, 128, 512], F32R)
        y_rt = dram.tile([HL, 128, T], F32R)

        psA = top.enter_context(tc.tile_pool(name="psA", bufs=4, space="PSUM"))
        psY = top.enter_context(tc.tile_pool(name="psY", bufs=2, space="PSUM"))
        psD = top.enter_context(tc.tile_pool(name="psD", bufs=2, space="PSUM"))

        const = top.enter_context(tc.tile_pool(name="const", bufs=1))
        ones_sb = const.tile([128, 128], F32R)
        nc.sync.dma_start(out=ones_sb, in_=ones_d[:, :])

        rep_ctx = tc.For_i(0, reps, 1) if reps > 1 else None
        if rep_ctx is not None:
            top.enter_context(rep_ctx)

        # ---------------- P1: projections + rope ----------------
        # xT resident (128KB/part); weights loaded once per head/group.
        with ExitStack() as p1x:
            xp = p1x.enter_context(tc.tile_pool(name="xp", bufs=1))
            xT_sb = xp.tile([128, NCT, T], F32R)
            for ct in range(NCT):
                nc.sync.dma_start(
                    out=xT_sb[:, ct, :],
                    in_=xT_d[ct * 128 : (ct + 1) * 128, :],
                )

            # v first (frees wv pool before qk weights arrive)
            with ExitStack() as p1v:
                wvp = p1v.enter_context(tc.tile_pool(name="wvp", bufs=1))
                bvp = p1v.enter_context(tc.tile_pool(name="bvp", bufs=1))
                ev0 = p1v.enter_context(tc.tile_pool(name="ev0", bufs=3))
                bv_sb = bvp.tile([128, CL], F32)
                nc.sync.dma_start(out=bv_sb, in_=bv_d[:].partition_broadcast(128))
                for g in range(NG):
                    gs = slice(g * 512, (g + 1) * 512)
                    wv_sb = wvp.tile([128, NCT, 512], F32R)
                    nc.sync.dma_start(
                        out=wv_sb,
                        in_=wv_d[:, gs].rearrange("(ct p) d -> p ct d", p=128),
                    )
                    for tt in range(NT):
                        xl = xT_sb[:, :, tt * 128 : (tt + 1) * 128]
                        ps = psA.tile([128, 512], F32, tag="mm")
                        for ct in range(NCT):
                            nc.tensor.matmul(
                                ps[:],
                                xl[:, ct, :],
                                wv_sb[:, ct, :],
                                start=(ct == 0),
                                stop=(ct == NCT - 1),
                            )
                        vt = ev0.tile([128, 512], F32R, tag="vt")
                        nc.vector.tensor_tensor(vt[:], ps[:], bv_sb[:, gs], op=ALU.add)
                        nc.sync.dma_start(out=v_rt[tt, g], in_=vt[:])

            # q/k per head, all four t-slices per weight load
            with ExitStack() as p1qk:
                wqk = p1qk.enter_context(tc.tile_pool(name="wqk", bufs=2))
                ev1 = p1qk.enter_context(tc.tile_pool(name="ev1", bufs=2))
                trig = p1qk.enter_context(tc.tile_pool(name="trig", bufs=1))
                cos_sb = trig.tile([128, T], F32)
                sin_sb = trig.tile([128, T], F32)
                nc.sync.dma_start(out=cos_sb, in_=cos_d[:, :])
                nc.sync.dma_start(out=sin_sb, in_=sin_d[:, :])
                bq_sb = trig.tile([128, HL], F32)
                bk_sb = trig.tile([128, HL], F32)
                nc.sync.dma_start(out=bq_sb, in_=bq_d.rearrange("(h p) -> p h", p=128))
                nc.sync.dma_start(out=bk_sb, in_=bk_d.rearrange("(h p) -> p h", p=128))

                for h in range(HL):
                    hs = slice(h * 128, (h + 1) * 128)
                    for w_d, b_sb, o_rt, wtag in (
                        (wq_d, bq_sb, q_rt, "wqs"),
                        (wk_d, bk_sb, k_rt, "wks"),
                    ):
                        w_sb = wqk.tile([128, NCT, 128], F32R, tag=wtag)
                        nc.sync.dma_start(
                            out=w_sb,
                            in_=w_d[:, hs].rearrange("(ct p) d -> p ct d", p=128),
                        )
                        for s in range(NQ):
                            ts = slice(s * 512, (s + 1) * 512)
                            ps = psA.tile([128, 512], F32, tag="mm")
                            for ct in range(NCT):
                                nc.tensor.matmul(
                                    ps[:],
                                    w_sb[:, ct, :],
                                    xT_sb[:, ct, ts],
                                    start=(ct == 0),
                                    stop=(ct == NCT - 1),
                                )
                            raw = ev1.tile([128, 512], F32, tag="qkraw")
                            nc.scalar.activation(
                                out=raw[:], in_=ps[:], func=AF.Identity,
                                bias=b_sb[:, h : h + 1], scale=1.0,
                            )
                            p1t = ev1.tile([128, 512], F32, tag="p1t")
                            p2t = ev1.tile([128, 512], F32, tag="p2t")
                            nc.vector.tensor_tensor(p1t[:], raw[:], cos_sb[:, ts], op=ALU.mult)
                            nc.vector.tensor_tensor(p2t[:], raw[:], sin_sb[:, ts], op=ALU.mult)
                            swp = ev1.tile([128, 512], F32, tag="swp")
                            nc.sync.dma_start(out=swp[0:64, :], in_=p2t[64:128, :])
                            nc.sync.dma_start(out=swp[64:128, :], in_=p2t[0:64, :])
                            rot = ev1.tile([128, 512], F32R, tag="rot")
                            nc.vector.tensor_tensor(rot[:], p1t[:], swp[:], op=ALU.add)
                            nc.sync.dma_start(out=o_rt[h, :, ts], in_=rot[:])

        # ---------------- P2: attention per head ----------------
        wop_ctx = ExitStack()
        wop = wop_ctx.enter_context(tc.tile_pool(name="wop", bufs=1))
        wo_sb = wop.tile([128, HL, NCQ, 512], F32R)
        nc.sync.dma_start(
            out=wo_sb,
            in_=wo_d.rearrange("(h p) (cq c) -> p h cq c", p=128, c=512),
        )
        with ExitStack() as p2:
            qkp = p2.enter_context(tc.tile_pool(name="qkp", bufs=2))
            vhp = p2.enter_context(tc.tile_pool(name="vhp", bufs=2))
            ep = p2.enter_context(tc.tile_pool(name="ep", bufs=6))
            yp = p2.enter_context(tc.tile_pool(name="yp", bufs=3))
            rp = p2.enter_context(tc.tile_pool(name="rp", bufs=3))
            mp = p2.enter_context(tc.tile_pool(name="mp", bufs=1))

            masks_sb = mp.tile([128, 4, 512], F32)
            nc.sync.dma_start(out=masks_sb, in_=mask_d.rearrange("m p f -> p m f"))

            for h in range(HL):
                g, off = h // 4, (h % 4) * 128
                q_sb = qkp.tile([128, T], F32R, tag="qh")
                k_sb = qkp.tile([128, T], F32R, tag="kh")
                v_sb = vhp.tile([128, NT, 128], F32R)
                nc.sync.dma_start(out=q_sb, in_=q_rt[h])
                nc.sync.dma_start(out=k_sb, in_=k_rt[h])
                nc.sync.dma_start(
                    out=v_sb,
                    in_=v_rt[:, g, :, off : off + 128].rearrange("n p d -> p n d"),
                )
                for j in range(NQ):
                    js = slice(j * 512, (j + 1) * 512)
                    nblk = 4 * (j + 1)
                    psd = psD.tile([128, 512], F32)
                    psy = psY.tile([128, 512], F32)
                    # software-pipelined: St/E one block ahead of den/y
                    etiles = []
                    for b in range(nblk):
                        pss = psA.tile([128, 512], F32, tag="mm")
                        nc.tensor.matmul(
                            pss[:],
                            r32(k_sb[:, b * 128 : (b + 1) * 128]),
                            r32(q_sb[:, js]),
                            start=True,
                            stop=True,
                        )
                        et = ep.tile([128, 512], F32R)
                        nc.scalar.activation(
                            out=et[:], in_=pss[:], func=AF.Exp, scale=scale
                        )
                        if b >= 4 * j:
                            nc.vector.tensor_tensor(
                                et[:], et[:], masks_sb[:, b - 4 * j, :], op=ALU.mult
                            )
                        etiles.append(et)
                        if b >= 1:
                            eprev = etiles[b - 1]
                            nc.tensor.matmul(
                                psd[:], r32(ones_sb[:]), r32(eprev[:]),
                                start=(b == 1), stop=False,
                            )
                            nc.tensor.matmul(
                                psy[:], r32(v_sb[:, b - 1, :]), r32(eprev[:]),
                                start=(b == 1), stop=False,
                            )
                    elast = etiles[nblk - 1]
                    nc.tensor.matmul(
                        psd[:], r32(ones_sb[:]), r32(elast[:]),
                        start=(nblk == 1), stop=True,
                    )
                    nc.tensor.matmul(
                        psy[:], r32(v_sb[:, nblk - 1, :]), r32(elast[:]),
                        start=(nblk == 1), stop=True,
                    )
                    recb = rp.tile([128, 512], F32, tag="recb")
                    nc.vector.reciprocal(out=recb[:], in_=psd[:])
                    yt = yp.tile([128, 512], F32R)
                    nc.vector.tensor_tensor(yt[:], psy[:], recb[:], op=ALU.mult)
                    nc.sync.dma_start(out=y_rt[h, :, js], in_=yt[:])

        # ---------------- P3: output projection ----------------
        with ExitStack() as p3:
            y3p = p3.enter_context(tc.tile_pool(name="y3p", bufs=3))
            op = p3.enter_context(tc.tile_pool(name="op", bufs=3))

            for tt in range(NT):
                tsl = slice(tt * 128, (tt + 1) * 128)
                yts = y3p.tile([128, HL, 128], F32R)
                nc.sync.dma_start(
                    out=yts, in_=y_rt[:, :, tsl].rearrange("h p t -> p h t")
                )
                for cq in range(NCQ):
                    ps = psA.tile([128, 512], F32, tag="mm")
                    for h in range(HL):
                        nc.tensor.matmul(
                            ps[:],
                            r32(yts[:, h, :]),
                            r32(wo_sb[:, h, cq, :]),
                            start=(h == 0),
                            stop=(h == HL - 1),
                        )
                    ot = op.tile([128, 512], F32)
                    nc.scalar.copy(out=ot[:], in_=ps[:])
                    nc.sync.dma_start(
                        out=out_d[tsl, cq * 512 : (cq + 1) * 512], in_=ot[:]
                    )
        wop_ctx.close()

    nc.finalize()
    return nc


def _col_perm(CL):
    """Per-head even/odd de-interleave of columns."""
    perm = []
    for h in range(CL // 128):
        base = h * 128
        perm += [base + i for i in range(0, 128, 2)]
        perm += [base + i for i in range(1, 128, 2)]
    return np.array(perm)


def host_prepare(x, Wq, bq, Wk, bk, Wv, bv, Wo, bo, T=None):
    """Build the 8 per-core input maps. x: [B, T, C] fp32."""
    B, Tfull, C = x.shape
    if T is None:
        T = Tfull
    D = HEAD_DIM
    perm = _col_perm(C)
    Wq_p = np.ascontiguousarray(Wq[:, perm])
    Wk_p = np.ascontiguousarray(Wk[:, perm])
    bq_p = np.ascontiguousarray(bq[perm])
    bk_p = np.ascontiguousarray(bk[perm])

    # rope tables, fp32 to mirror the reference computation
    inv = (1.0 / (ROPE_BASE ** (np.arange(0, D, 2, dtype=np.float32) / D))).astype(
        np.float32
    )
    pos = np.arange(T, dtype=np.float32)
    th = pos[None, :] * inv[:, None]          # [64, T]
    cos1 = np.cos(th).astype(np.float32)
    sin1 = np.sin(th).astype(np.float32)
    cos2 = np.concatenate([cos1, cos1], axis=0)
    sin2 = np.concatenate([sin1, -sin1], axis=0)

    m = np.zeros((4, 128, 512), dtype=np.float32)
    p = np.arange(128)[:, None]
    f = np.arange(512)[None, :]
    for mi in range(4):
        m[mi] = ((p + mi * 128) <= f).astype(np.float32)

    in_maps = []
    for core in range(8):
        b, half = core // 2, core % 2
        cl = slice(half * 1024, (half + 1) * 1024)
        xT = np.ascontiguousarray(x[b, :T].T)
        in_maps.append(
            {
                "xT": xT,
                "wq": np.ascontiguousarray(Wq_p[:, cl]),
                "wk": np.ascontiguousarray(Wk_p[:, cl]),
                "wv": np.ascontiguousarray(Wv[:, cl]),
                "wo": np.ascontiguousarray(Wo[cl.start : cl.stop, :]),
                "bq": np.ascontiguousarray(bq_p[cl]),
                "bk": np.ascontiguousarray(bk_p[cl]),
                "bv": np.ascontiguousarray(bv[cl]),
                "cos2": cos2,
                "sin2": sin2,
                "masks": m,
                "ones": np.ones((128, 128), dtype=np.float32),
            }
        )
    return in_maps


def assemble(results, bo, B, T, C):
    out = np.empty((B, T, C), dtype=np.float32)
    for b in range(B):
        out[b] = results[2 * b]["out_p"] + results[2 * b + 1]["out_p"] + bo[None, :]
    return out


# ---------------------------------------------------------------------------
# SPMD execution via PJRT/axon (compiles once per process, reusable)
# ---------------------------------------------------------------------------
import jax
from jax.sharding import Mesh, PartitionSpec
from jax.experimental.shard_map import shard_map

from concourse.bass2jax import (
    _bass_exec_p,
    install_neuronx_cc_hook,
    partition_id_tensor,
)


class _SpmdRunner:
    def __init__(self, nc, n_cores):
        install_neuronx_cc_hook()
        self.nc = nc
        self.n_cores = n_cores
        partition_name = (
            nc.partition_id_tensor.name if nc.partition_id_tensor else None
        )
        in_names, out_names, out_avals, zero_outs = [], [], [], []
        for alloc in nc.m.functions[0].allocations:
            if not isinstance(alloc, mybir.MemoryLocationSet):
                continue
            name = alloc.memorylocations[0].name
            if alloc.kind == "ExternalInput":
                if name != partition_name:
                    in_names.append(name)
            elif alloc.kind == "ExternalOutput":
                shape = tuple(alloc.tensor_shape)
                dtype = mybir.dt.np(alloc.dtype)
                out_names.append(name)
                out_avals.append(jax.core.ShapedArray(shape, dtype))
                zero_outs.append(np.zeros(shape, dtype))
        n_params = len(in_names)
        all_in_names = list(in_names) + list(out_names)
        if partition_name is not None:
            all_in_names.append(partition_name)
        self.in_names, self.out_names = in_names, out_names
        self.out_avals, self.zero_outs = out_avals, zero_outs

        def _body(*args):
            operands = list(args)
            if partition_name is not None:
                operands.append(partition_id_tensor())
            outs = _bass_exec_p.bind(
                *operands,
                out_avals=tuple(out_avals),
                in_names=tuple(all_in_names),
                out_names=tuple(out_names),
                lowering_input_output_aliases=(),
                sim_require_finite=True,
                sim_require_nnan=True,
                nc=nc,
            )
            return tuple(outs)

        devices = jax.devices()[:n_cores]
        assert len(devices) == n_cores, (
            f"need {n_cores} neuron cores, found {len(jax.devices())}"
        )
        mesh = Mesh(np.asarray(devices), ("core",))
        n_outs = len(out_avals)
        self.sharding = jax.sharding.NamedSharding(mesh, PartitionSpec("core"))
        self.fn = jax.jit(
            shard_map(
                _body,
                mesh=mesh,
                in_specs=(PartitionSpec("core"),) * (n_params + n_outs),
                out_specs=(PartitionSpec("core"),) * n_outs,
                check_rep=False,
            ),
            keep_unused=True,
        )

    def run(self, in_maps):
        n = self.n_cores
        concat_in = [
            np.concatenate(
                [np.asarray(in_maps[c][name]) for c in range(n)], axis=0
            )
            for name in self.in_names
        ]
        concat_zero = [
            np.zeros((n * z.shape[0], *z.shape[1:]), z.dtype)
            for z in self.zero_outs
        ]
        out_arrs = self.fn(*concat_in, *concat_zero)
        jax.block_until_ready(out_arrs)
        return [
            {
                name: np.asarray(out_arrs[i]).reshape(
                    n, *self.out_avals[i].shape
                )[c]
                for i, name in enumerate(self.out_names)
            }
            for c in range(n)
        ]


_RUNNER_CACHE = {}


def _get_runner(reps=1):
    key = reps
    if key not in _RUNNER_CACHE:
        nc = build_core_kernel(T=2048, C=2048, HL=8, reps=reps, pool_mode="queue")
        _RUNNER_CACHE[key] = _SpmdRunner(nc, 8)
    return _RUNNER_CACHE[key]


def kernel(x, Wq, bq, Wk, bk, Wv, bv, Wo, bo, _reps=1):
    """Causal self-attention with RoPE. Full inputs in, full output out.

    Shards batch (4) x head-halves (2) across the 8 NeuronCores; each
    core computes a partial [T, C] output; core pairs are summed on the
    host (the tensor-parallel all-reduce) and bo is added.
    """
    x = np.ascontiguousarray(np.asarray(x, dtype=np.float32))
    B, T, C = x.shape
    in_maps = host_prepare(
        np.asarray(x), np.asarray(Wq), np.asarray(bq), np.asarray(Wk),
        np.asarray(bk), np.asarray(Wv), np.asarray(bv), np.asarray(Wo),
        np.asarray(bo),
    )
    runner = _get_runner(_reps)
    results = runner.run(in_maps)
    return assemble(results, np.asarray(bo, dtype=np.float32), B, T, C)
